# revision 15
# baseline (speedup 1.0000x reference)
"""Trainium2 Bass kernel for nn_DirectionalCurvatureLoss.

Computes reference():
  5-stencil 3x3 conv (sobel_x/y, k_xx/xy/yy) on pred & target ->
  profile/planform/mean curvature -> weighted mean-abs-diff scalar loss.

Strategy (8 cores, pure data parallel over batch B=16 -> 2 images/core):
  - Separable stencils: horizontal 3-tap pass on DVE (free axis, bf16 for
    the 2x perf mode), vertical 3-tap pass as banded-matrix matmuls on the
    TensorEngine (contracts over the partition axis; integer bf16 bands,
    fp32 PSUM accumulate; per-filter scales folded into elementwise consts).
  - Curvature algebra on DVE via fused scalar_tensor_tensor bf16 ops; the
    denominators go through a fp32 Ln/Exp chain on the ScalarEngine (single
    act-table set `natural_log_exp_and_others`, so no table switches):
    1/(x+eps) = exp(-ln(x+eps)), d1^1.5 = exp(1.5*ln(d1+delta)),
    d1*sqrt(1+d1) = exp(ln(d1+delta) + 0.5*ln(1+d1)).
  - |pred-target| + free-axis reduction fused into ACT Abs(accum_out).
  - Per-core partial sums [128, 64] f32 DMA'd out; host does the final
    (tiny) reduction in float64 and applies the loss weights / N.

Numerics: bf16 rounding noise averages out in the 16.7M-pixel mean;
validated ~2e-4 relative loss error vs the fp32 reference pipeline.
"""

import numpy as np

H = 1024
W = 1024
B = 16
N_CORES = 8
B_PER_CORE = B // N_CORES  # 2
EPS = 1e-8
DELTA = 1e-37
W_PROFILE, W_PLANFORM, W_MEAN = 0.5, 0.3, 0.2

# block geometry: (i0, K, o0, M): input rows [i0, i0+K), valid output rows [o0, o0+M)
_BLOCKS = [(0, 128, 0, 127)]
_BLOCKS += [(127 + 126 * j - 1, 128, 127 + 126 * j, 126) for j in range(7)]
_BLOCKS += [(1008, 16, 1009, 15)]
assert _BLOCKS[-1][2] + _BLOCKS[-1][3] == H

_VERTS = {"a": (1, 2, 1), "b": (-1, 0, 1), "c": (1, 1, 1), "d": (1, -2, 1)}

N_PAIR_UNITS = B_PER_CORE * len(_BLOCKS)  # 18 loss units -> 54 accum columns
ACC_COLS = 64

_BAND_OFF = {}
_off = 0
for _name in ("a", "b", "c", "d"):
    for _kind, _w in (("first", 127), ("mid", 126)):
        _BAND_OFF[(_name, _kind)] = (_off, _w)
        _off += _w
BANDS_W = _off  # 1012
# constant columns (fp32 sidecar tensor): DELTA, 1.0, EPS, ln(0.5)
_CONST_COL = {"delta": 0, "one": 1, "eps": 2, "lnhalf": 3}
CONSTS_W = 4


def _make_bands() -> np.ndarray:
    """[128, 1012] bf16: per vert in a,b,c,d: A_first [128,127], A_mid [128,126].

    A_first[k, m] = vert[k - m + 1]  (block 0: i0 == o0)
    A_mid[k, m]   = vert[k - m]      (i0 == o0 - 1; sliced for the last block)
    Integer taps -> exact in bf16.
    """
    import ml_dtypes

    segs = []
    for name in ("a", "b", "c", "d"):
        v = _VERTS[name]
        for kind in ("first", "mid"):
            Mw = 127 if kind == "first" else 126
            A = np.zeros((128, Mw), np.float32)
            k = np.arange(128)[:, None]
            m = np.arange(Mw)[None, :]
            d = k - m + (1 if kind == "first" else 0)
            mask = (d >= 0) & (d <= 2)
            A[mask] = np.asarray(v, np.float32)[d[mask]]
            segs.append(A)
    return np.concatenate(segs, axis=1).astype(ml_dtypes.bfloat16)


def _make_consts() -> np.ndarray:
    c = np.zeros((128, CONSTS_W), np.float32)
    c[:, 0] = DELTA
    c[:, 1] = 1.0
    c[:, 2] = EPS
    c[:, 3] = np.log(0.5)
    return c


def _build_nc():
    import concourse.bass as bass
    import concourse.mybir as mybir
    from concourse import tile

    f32 = mybir.dt.float32
    bf16 = mybir.dt.bfloat16
    Alu = mybir.AluOpType
    Act = mybir.ActivationFunctionType

    nc = bass.Bass()
    pred_d = nc.dram_tensor("pred", [B_PER_CORE, 1, H, W], f32, kind="ExternalInput")
    targ_d = nc.dram_tensor("target", [B_PER_CORE, 1, H, W], f32, kind="ExternalInput")
    bands_d = nc.dram_tensor("bands", [128, BANDS_W], bf16, kind="ExternalInput")
    consts_d = nc.dram_tensor("consts", [128, CONSTS_W], f32, kind="ExternalInput")
    acc_d = nc.dram_tensor("acc", [128, ACC_COLS], f32, kind="ExternalOutput")

    with tile.TileContext(nc) as tc:
        with (
            tc.tile_pool(name="persist", bufs=1) as persist,
            tc.tile_pool(name="xin", bufs=2) as xin,
            tc.tile_pool(name="hp", bufs=2) as hp,
            tc.tile_pool(name="cv", bufs=2) as cv,
            tc.tile_pool(name="num", bufs=2) as num,
            tc.tile_pool(name="den", bufs=2) as den,
            tc.tile_pool(name="outp", bufs=2) as outp,
            tc.tile_pool(name="psum", bufs=4, space=bass.MemorySpace.PSUM) as psum,
        ):
            bands = persist.tile([128, BANDS_W], bf16)
            consts = persist.tile([128, CONSTS_W], f32)
            accbuf = persist.tile([128, ACC_COLS], f32)
            nc.sync.dma_start(bands[:], bands_d[:])
            nc.sync.dma_start(consts[:], consts_d[:])
            nc.gpsimd.memset(accbuf[:], 0.0)

            def band_ap(name, kind, K, M):
                off, w = _BAND_OFF[(name, kind)]
                assert M <= w
                return bands[0:K, off : off + M]

            def cbias(name, M):
                c = _CONST_COL[name]
                return consts[0:M, c : c + 1]

            def unit(img_d, s, blk):
                """one image x one row block -> (prof, plan, mean) bf16 tiles"""
                i0, K, o0, M = blk
                kind = "first" if i0 == o0 else "mid"

                Xt = xin.tile([128, W + 2], f32, tag="X")
                nc.sync.dma_start(Xt[0:K, 1 : W + 1], img_d[s, 0, i0 : i0 + K, :])
                nc.vector.memset(Xt[0:K, 0:1], 0.0)
                nc.vector.memset(Xt[0:K, W + 1 : W + 2], 0.0)

                # bf16 casts: Xb (even-offset taps 0/2), Xm (middle tap),
                # Xm2 = 2*middle (all offset-0 aligned -> every H op is 2x TT)
                Xb = xin.tile([128, W + 2], bf16, tag="Xb")
                Xm = xin.tile([128, W], bf16, tag="Xm")
                Xm2 = xin.tile([128, W], bf16, tag="Xm2")
                nc.vector.tensor_copy(Xb[0:K], Xt[0:K])
                nc.vector.tensor_copy(Xm[0:K], Xt[0:K, 1 : W + 1])
                nc.vector.tensor_scalar_mul(Xm2[0:K], Xt[0:K, 1 : W + 1], 2.0)

                t1 = hp.tile([128, W], bf16, tag="t1")
                ha = hp.tile([128, W], bf16, tag="ha")
                hb = hp.tile([128, W], bf16, tag="hb")
                hc = hp.tile([128, W], bf16, tag="hc")
                hd = hp.tile([128, W], bf16, tag="hd")
                nc.vector.tensor_tensor(
                    t1[0:K], Xb[0:K, 0:W], Xb[0:K, 2 : W + 2], Alu.add
                )
                nc.vector.tensor_tensor(
                    hb[0:K], Xb[0:K, 2 : W + 2], Xb[0:K, 0:W], Alu.subtract
                )
                nc.vector.tensor_tensor(ha[0:K], t1[0:K], Xm2[0:K], Alu.add)
                nc.vector.tensor_tensor(hc[0:K], t1[0:K], Xm2[0:K], Alu.subtract)
                nc.vector.tensor_tensor(hd[0:K], Xm[0:K], t1[0:K], Alu.add)

                # vertical convs on PE: out[m, w] = sum_k A[k, m] * h[k, w]
                pP = psum.tile([128, W], f32, tag="cv")
                pQ = psum.tile([128, W], f32, tag="cv")
                pR = psum.tile([128, W], f32, tag="cv")
                pS = psum.tile([128, W], f32, tag="cv")
                pT = psum.tile([128, W], f32, tag="cv")
                pRT = psum.tile([128, W], f32, tag="cv")

                def mm(out_t, vname, h_t, start=True, stop=True):
                    lhsT = band_ap(vname, kind, K, M)
                    for wo in (0, 512):
                        nc.tensor.matmul(
                            out_t[0:M, wo : wo + 512],
                            lhsT,
                            h_t[0:K, wo : wo + 512],
                            start=start,
                            stop=stop,
                        )

                # order for stationary reuse: c (R, RT), d (T, RT), a (P), b (Q, S)
                mm(pR, "c", hc)
                mm(pRT, "c", hc, start=True, stop=False)
                mm(pT, "d", hd)
                mm(pRT, "d", hd, start=False, stop=True)
                mm(pP, "a", hb)
                mm(pQ, "b", ha)
                mm(pS, "b", hb)

                # evacuate all 6 conv outputs to bf16 SBUF (ACT copies)
                Pb = cv.tile([128, W], bf16, tag="Pb")
                Qb = cv.tile([128, W], bf16, tag="Qb")
                Rb = cv.tile([128, W], bf16, tag="Rb")
                Sb = cv.tile([128, W], bf16, tag="Sb")
                Tb = cv.tile([128, W], bf16, tag="Tb")
                RTb = cv.tile([128, W], bf16, tag="RTb")
                # evac with the per-filter scales folded in -> true p,q,r,2s,t,rt
                nc.scalar.mul(Pb[0:M], pP[0:M], 1.0 / 80)
                nc.scalar.mul(Qb[0:M], pQ[0:M], 1.0 / 80)
                nc.scalar.mul(Rb[0:M], pR[0:M], 1.0 / 300)
                nc.scalar.mul(Sb[0:M], pS[0:M], 2.0 / 400)
                nc.scalar.mul(Tb[0:M], pT[0:M], 1.0 / 300)
                nc.scalar.mul(RTb[0:M], pRT[0:M], 1.0 / 300)

                p2 = cv.tile([128, W], bf16, tag="p2")
                q2 = cv.tile([128, W], bf16, tag="q2")
                pq = cv.tile([128, W], bf16, tag="pq")
                d1 = cv.tile([128, W], bf16, tag="d1")
                nc.scalar.activation(p2[0:M], Pb[0:M], Act.Square)
                nc.scalar.activation(q2[0:M], Qb[0:M], Act.Square)
                nc.vector.tensor_tensor(pq[0:M], Pb[0:M], Qb[0:M], Alu.mult)
                nc.vector.tensor_tensor(d1[0:M], p2[0:M], q2[0:M], Alu.add)

                AB1 = num.tile([128, W], bf16, tag="AB1")
                AB2 = num.tile([128, W], bf16, tag="AB2")
                Un = num.tile([128, W], bf16, tag="Un")
                Vn = num.tile([128, W], bf16, tag="Vn")
                N1 = num.tile([128, W], bf16, tag="N1")
                rtd1 = num.tile([128, W], bf16, tag="rtd1")
                PRn = num.tile([128, W], bf16, tag="PRn")
                MNn = num.tile([128, W], bf16, tag="MNn")
                nc.vector.tensor_tensor(AB1[0:M], Rb[0:M], q2[0:M], Alu.mult)
                nc.vector.tensor_tensor(AB2[0:M], Tb[0:M], p2[0:M], Alu.mult)
                nc.vector.tensor_tensor(Un[0:M], AB1[0:M], AB2[0:M], Alu.add)
                nc.vector.tensor_tensor(Vn[0:M], Sb[0:M], pq[0:M], Alu.mult)
                nc.vector.tensor_tensor(N1[0:M], Un[0:M], Vn[0:M], Alu.add)
                nc.vector.tensor_tensor(rtd1[0:M], RTb[0:M], d1[0:M], Alu.mult)
                nc.vector.tensor_tensor(PRn[0:M], rtd1[0:M], N1[0:M], Alu.subtract)
                nc.vector.tensor_tensor(MNn[0:M], RTb[0:M], N1[0:M], Alu.add)

                # denominators: fp32 Ln/Exp chain on ACT
                ul = den.tile([128, W], f32, tag="ul")
                vl = den.tile([128, W], f32, tag="vl")
                lnm1 = den.tile([128, W], f32, tag="lnm1")
                m1x = den.tile([128, W], f32, tag="m1x")
                Lp = den.tile([128, W], f32, tag="Lp")
                invp = den.tile([128, W], bf16, tag="invp")
                d15x = den.tile([128, W], f32, tag="d15x")
                Ll = den.tile([128, W], f32, tag="Ll")
                invl = den.tile([128, W], bf16, tag="invl")
                invm = den.tile([128, W], bf16, tag="invm")
                nc.scalar.activation(ul[0:M], d1[0:M], Act.Ln, bias=cbias("delta", M))
                nc.scalar.activation(vl[0:M], d1[0:M], Act.Ln, bias=cbias("one", M))
                nc.vector.scalar_tensor_tensor(
                    lnm1[0:M], vl[0:M], 0.5, ul[0:M], Alu.mult, Alu.add
                )
                nc.scalar.activation(m1x[0:M], lnm1[0:M], Act.Exp)
                nc.scalar.activation(Lp[0:M], m1x[0:M], Act.Ln, bias=cbias("eps", M))
                nc.scalar.activation(invp[0:M], Lp[0:M], Act.Exp, scale=-1.0)
                nc.scalar.activation(d15x[0:M], ul[0:M], Act.Exp, scale=1.5)
                nc.scalar.activation(Ll[0:M], d15x[0:M], Act.Ln, bias=cbias("eps", M))
                nc.scalar.activation(invl[0:M], Ll[0:M], Act.Exp, scale=-1.0)
                nc.scalar.activation(
                    invm[0:M], vl[0:M], Act.Exp, scale=-1.5, bias=cbias("lnhalf", M)
                )

                prof = outp.tile([128, W], bf16, tag="prof")
                plan = outp.tile([128, W], bf16, tag="plan")
                mean = outp.tile([128, W], bf16, tag="mean")
                nc.vector.tensor_tensor(prof[0:M], PRn[0:M], invp[0:M], Alu.mult)
                nc.vector.tensor_tensor(plan[0:M], N1[0:M], invl[0:M], Alu.mult)
                nc.vector.tensor_tensor(mean[0:M], MNn[0:M], invm[0:M], Alu.mult)
                return prof, plan, mean

            pair_idx = 0
            for s in range(B_PER_CORE):
                for blk in _BLOCKS:
                    M = blk[3]
                    outs_p = unit(pred_d, s, blk)
                    outs_t = unit(targ_d, s, blk)
                    for ci in range(3):
                        dp = outp.tile([128, W], bf16, tag="dp")
                        dead = outp.tile([128, W], bf16, tag="dp")
                        nc.vector.tensor_tensor(
                            dp[0:M], outs_p[ci][0:M], outs_t[ci][0:M], Alu.subtract
                        )
                        col = pair_idx * 3 + ci
                        nc.scalar.activation(
                            dead[0:M],
                            dp[0:M],
                            Act.Abs,
                            accum_out=accbuf[0:M, col : col + 1],
                        )
                    pair_idx += 1

            nc.sync.dma_start(acc_d[:], accbuf[:])

    _split_multi_waits(nc, mybir)
    return nc


def _split_multi_waits(nc, mybir):
    """This environment's walrus accepts only ONE semaphore wait per
    instruction (ISA Events has a single wait slot); Tile fuses several.
    Split every multi-wait instruction into standalone single-wait
    EventSemaphore ops on the same engine queue (raw-bass wait_ge style),
    keeping the last wait + all updates on the real instruction."""
    cnt = 0
    for f in nc.m.functions:
        for blk in f.blocks:
            new_list = []
            for ins in blk.instructions:
                si = ins.sync_info
                waits = list(si.on_wait) if si is not None else []
                if len(waits) > 1:
                    for w in waits[:-1]:
                        cnt += 1
                        nop = mybir.InstEventSemaphore(
                            name=f"wsplit-{cnt}", ins=[], outs=[]
                        )
                        nop.engine = ins.engine
                        nop.sync_info = mybir.SyncInfo(on_wait=[w], on_update=[])
                        new_list.append(nop)
                    si.on_wait = [waits[-1]]
                new_list.append(ins)
            blk.instructions = new_list


_NC_CACHE = {}


def _get_nc():
    if "nc" not in _NC_CACHE:
        _NC_CACHE["nc"] = _build_nc()
    return _NC_CACHE["nc"]


def kernel(pred, target):
    pred = np.ascontiguousarray(np.asarray(pred), dtype=np.float32)
    target = np.ascontiguousarray(np.asarray(target), dtype=np.float32)
    assert pred.shape == (B, 1, H, W) and target.shape == (B, 1, H, W)

    from concourse.bass_utils import run_bass_kernel_spmd

    nc = _get_nc()
    bands = _make_bands()
    consts = _make_consts()
    in_maps = [
        {
            "pred": pred[c * B_PER_CORE : (c + 1) * B_PER_CORE],
            "target": target[c * B_PER_CORE : (c + 1) * B_PER_CORE],
            "bands": bands,
            "consts": consts,
        }
        for c in range(N_CORES)
    ]
    res = run_bass_kernel_spmd(nc, in_maps, list(range(N_CORES)))
    _NC_CACHE["last_results"] = res

    tot = np.zeros(3, np.float64)
    for c in range(N_CORES):
        acc = np.asarray(res.results[c]["acc"], np.float64)
        for ci in range(3):
            tot[ci] += acc[:, ci : N_PAIR_UNITS * 3 : 3].sum()
    n = float(B * H * W)
    loss = (W_PROFILE * tot[0] + W_PLANFORM * tot[1] + W_MEAN * tot[2]) / n
    return np.float32(loss)


# revision 25
# speedup vs baseline: 1.0910x; 1.0910x over previous
"""Trainium2 Bass kernel for nn_DirectionalCurvatureLoss.

Computes reference():
  5-stencil 3x3 conv (sobel_x/y, k_xx/xy/yy) on pred & target ->
  profile/planform/mean curvature -> weighted mean-abs-diff scalar loss.

Strategy (8 cores, pure data parallel over batch B=16 -> 2 images/core):
  - Separable stencils: horizontal 3-tap pass on DVE (free axis, bf16 for
    the 2x perf mode), vertical 3-tap pass as banded-matrix matmuls on the
    TensorEngine (contracts over the partition axis; integer bf16 bands,
    fp32 PSUM accumulate; per-filter scales folded into elementwise consts).
  - Curvature algebra on DVE via fused scalar_tensor_tensor bf16 ops; the
    denominators go through a fp32 Ln/Exp chain on the ScalarEngine (single
    act-table set `natural_log_exp_and_others`, so no table switches):
    1/(x+eps) = exp(-ln(x+eps)), d1^1.5 = exp(1.5*ln(d1+delta)),
    d1*sqrt(1+d1) = exp(ln(d1+delta) + 0.5*ln(1+d1)).
  - |pred-target| + free-axis reduction fused into ACT Abs(accum_out).
  - Per-core partial sums [128, 64] f32 DMA'd out; host does the final
    (tiny) reduction in float64 and applies the loss weights / N.

Numerics: bf16 rounding noise averages out in the 16.7M-pixel mean;
validated ~2e-4 relative loss error vs the fp32 reference pipeline.
"""

import numpy as np

H = 1024
W = 1024
B = 16
N_CORES = 8
B_PER_CORE = B // N_CORES  # 2
EPS = 1e-8
DELTA = 1e-37
W_PROFILE, W_PLANFORM, W_MEAN = 0.5, 0.3, 0.2

# block geometry: (i0, K, o0, M): input rows [i0, i0+K), valid output rows [o0, o0+M)
_BLOCKS = [(0, 128, 0, 127)]
_BLOCKS += [(127 + 126 * j - 1, 128, 127 + 126 * j, 126) for j in range(7)]
_BLOCKS += [(1008, 16, 1009, 15)]
assert _BLOCKS[-1][2] + _BLOCKS[-1][3] == H

_VERTS = {"a": (1, 2, 1), "b": (-1, 0, 1), "c": (1, 1, 1), "d": (1, -2, 1)}

N_PAIR_UNITS = B_PER_CORE * len(_BLOCKS)  # 18 loss units -> 54 accum columns
ACC_COLS = 64

_BAND_OFF = {}
_off = 0
for _name in ("a", "b", "c", "d"):
    for _kind, _w in (("first", 127), ("mid", 126)):
        _BAND_OFF[(_name, _kind)] = (_off, _w)
        _off += _w
BANDS_W = _off  # 1012
# constant columns (fp32 sidecar tensor): DELTA, 1.0, EPS, ln(0.5)
_CONST_COL = {"delta": 0, "one": 1, "eps": 2, "lnhalf": 3}
CONSTS_W = 4


def _make_bands() -> np.ndarray:
    """[128, 1012] bf16: per vert in a,b,c,d: A_first [128,127], A_mid [128,126].

    A_first[k, m] = vert[k - m + 1]  (block 0: i0 == o0)
    A_mid[k, m]   = vert[k - m]      (i0 == o0 - 1; sliced for the last block)
    Integer taps -> exact in bf16.
    """
    import ml_dtypes

    segs = []
    for name in ("a", "b", "c", "d"):
        v = _VERTS[name]
        for kind in ("first", "mid"):
            Mw = 127 if kind == "first" else 126
            A = np.zeros((128, Mw), np.float32)
            k = np.arange(128)[:, None]
            m = np.arange(Mw)[None, :]
            d = k - m + (1 if kind == "first" else 0)
            mask = (d >= 0) & (d <= 2)
            A[mask] = np.asarray(v, np.float32)[d[mask]]
            segs.append(A)
    return np.concatenate(segs, axis=1).astype(ml_dtypes.bfloat16)


def _make_consts() -> np.ndarray:
    c = np.zeros((128, CONSTS_W), np.float32)
    c[:, 0] = DELTA
    c[:, 1] = 1.0
    c[:, 2] = EPS
    c[:, 3] = np.log(0.5)
    return c


def _build_nc(split=True):
    import concourse.bass as bass
    import concourse.mybir as mybir
    from concourse import tile

    f32 = mybir.dt.float32
    bf16 = mybir.dt.bfloat16
    Alu = mybir.AluOpType
    Act = mybir.ActivationFunctionType

    nc = bass.Bass()
    pred_d = nc.dram_tensor("pred", [B_PER_CORE, 1, H, W], f32, kind="ExternalInput")
    targ_d = nc.dram_tensor("target", [B_PER_CORE, 1, H, W], f32, kind="ExternalInput")
    bands_d = nc.dram_tensor("bands", [128, BANDS_W], bf16, kind="ExternalInput")
    consts_d = nc.dram_tensor("consts", [128, CONSTS_W], f32, kind="ExternalInput")
    acc_d = nc.dram_tensor("acc", [128, ACC_COLS], f32, kind="ExternalOutput")

    with tile.TileContext(nc) as tc:
        with (
            tc.tile_pool(name="persist", bufs=1) as persist,
            tc.tile_pool(name="xin", bufs=2) as xin,
            tc.tile_pool(name="hp", bufs=2) as hp,
            tc.tile_pool(name="cv", bufs=2) as cv,
            tc.tile_pool(name="num", bufs=2) as num,
            tc.tile_pool(name="den", bufs=2) as den,
            tc.tile_pool(name="outp", bufs=2) as outp,
            tc.tile_pool(name="psum", bufs=4, space=bass.MemorySpace.PSUM) as psum,
        ):
            bands = persist.tile([128, BANDS_W], bf16)
            consts = persist.tile([128, CONSTS_W], f32)
            accbuf = persist.tile([128, ACC_COLS], f32)
            nc.sync.dma_start(bands[:], bands_d[:])
            nc.sync.dma_start(consts[:], consts_d[:])
            nc.gpsimd.memset(accbuf[:], 0.0)

            def band_ap(name, kind, K, M):
                off, w = _BAND_OFF[(name, kind)]
                assert M <= w
                return bands[0:K, off : off + M]

            def cbias(name, M):
                c = _CONST_COL[name]
                return consts[0:M, c : c + 1]

            def unit(img_d, s, blk):
                """one image x one row block -> (prof, plan, mean) bf16 tiles"""
                i0, K, o0, M = blk
                kind = "first" if i0 == o0 else "mid"

                Xt = xin.tile([128, W + 2], f32, tag="X")
                nc.sync.dma_start(Xt[0:K, 1 : W + 1], img_d[s, 0, i0 : i0 + K, :])
                nc.vector.memset(Xt[0:K, 0:1], 0.0)
                nc.vector.memset(Xt[0:K, W + 1 : W + 2], 0.0)

                # bf16 casts: Xb (even-offset taps 0/2), Xm (middle tap),
                # Xm2 = 2*middle (all offset-0 aligned -> every H op is 2x TT)
                Xb = xin.tile([128, W + 2], bf16, tag="Xb")
                Xm = xin.tile([128, W], bf16, tag="Xm")
                Xm2 = xin.tile([128, W], bf16, tag="Xm2")
                nc.vector.tensor_copy(Xb[0:K], Xt[0:K])
                nc.vector.tensor_copy(Xm[0:K], Xt[0:K, 1 : W + 1])
                nc.gpsimd.tensor_scalar_mul(Xm2[0:K], Xt[0:K, 1 : W + 1], 2.0)

                t1 = hp.tile([128, W], bf16, tag="t1")
                ha = hp.tile([128, W], bf16, tag="ha")
                hb = hp.tile([128, W], bf16, tag="hb")
                hc = hp.tile([128, W], bf16, tag="hc")
                hd = hp.tile([128, W], bf16, tag="hd")
                nc.vector.tensor_tensor(
                    t1[0:K], Xb[0:K, 0:W], Xb[0:K, 2 : W + 2], Alu.add
                )
                nc.vector.tensor_tensor(
                    hb[0:K], Xb[0:K, 2 : W + 2], Xb[0:K, 0:W], Alu.subtract
                )
                nc.vector.tensor_tensor(ha[0:K], t1[0:K], Xm2[0:K], Alu.add)
                nc.vector.tensor_tensor(hc[0:K], t1[0:K], Xm2[0:K], Alu.subtract)
                nc.vector.tensor_tensor(hd[0:K], Xm[0:K], t1[0:K], Alu.add)

                # vertical convs on PE: out[m, w] = sum_k A[k, m] * h[k, w]
                pP = psum.tile([128, W], f32, tag="cv")
                pQ = psum.tile([128, W], f32, tag="cv")
                pR = psum.tile([128, W], f32, tag="cv")
                pS = psum.tile([128, W], f32, tag="cv")
                pT = psum.tile([128, W], f32, tag="cv")
                pRT = psum.tile([128, W], f32, tag="cv")

                def mm(out_t, vname, h_t, start=True, stop=True):
                    lhsT = band_ap(vname, kind, K, M)
                    for wo in (0, 512):
                        nc.tensor.matmul(
                            out_t[0:M, wo : wo + 512],
                            lhsT,
                            h_t[0:K, wo : wo + 512],
                            start=start,
                            stop=stop,
                        )

                # order for stationary reuse: c (R, RT), d (T, RT), a (P), b (Q, S)
                mm(pR, "c", hc)
                mm(pRT, "c", hc, start=True, stop=False)
                mm(pT, "d", hd)
                mm(pRT, "d", hd, start=False, stop=True)
                mm(pP, "a", hb)
                mm(pQ, "b", ha)
                mm(pS, "b", hb)

                # evacuate all 6 conv outputs to bf16 SBUF (ACT copies)
                Pb = cv.tile([128, W], bf16, tag="Pb")
                Qb = cv.tile([128, W], bf16, tag="Qb")
                Rb = cv.tile([128, W], bf16, tag="Rb")
                Tb = cv.tile([128, W], bf16, tag="Tb")
                RTb = cv.tile([128, W], bf16, tag="RTb")
                # evac with the per-filter scales folded in -> true p,q,r,2s,t,rt
                nc.scalar.mul(Pb[0:M], pP[0:M], 1.0 / 80)
                nc.scalar.mul(Qb[0:M], pQ[0:M], 1.0 / 80)
                nc.scalar.mul(Rb[0:M], pR[0:M], 1.0 / 300)
                nc.scalar.mul(Tb[0:M], pT[0:M], 1.0 / 300)
                nc.scalar.mul(RTb[0:M], pRT[0:M], 1.0 / 300)

                p2 = cv.tile([128, W], bf16, tag="p2")
                q2 = cv.tile([128, W], bf16, tag="q2")
                pq = cv.tile([128, W], bf16, tag="pq")
                d1 = cv.tile([128, W], bf16, tag="d1")
                nc.vector.tensor_tensor(p2[0:M], Pb[0:M], Pb[0:M], Alu.mult)
                nc.vector.tensor_tensor(q2[0:M], Qb[0:M], Qb[0:M], Alu.mult)
                nc.vector.tensor_tensor(pq[0:M], Pb[0:M], Qb[0:M], Alu.mult)
                nc.vector.tensor_tensor(d1[0:M], p2[0:M], q2[0:M], Alu.add)

                AB1 = num.tile([128, W], bf16, tag="AB1")
                AB2 = num.tile([128, W], bf16, tag="AB2")
                Un = num.tile([128, W], bf16, tag="Un")
                Vn = num.tile([128, W], bf16, tag="Vn")
                N1 = num.tile([128, W], bf16, tag="N1")
                rtd1 = num.tile([128, W], bf16, tag="rtd1")
                PRn = num.tile([128, W], bf16, tag="PRn")
                MNn = num.tile([128, W], bf16, tag="MNn")
                nc.vector.tensor_tensor(AB1[0:M], Rb[0:M], q2[0:M], Alu.mult)
                nc.vector.tensor_tensor(AB2[0:M], Tb[0:M], p2[0:M], Alu.mult)
                nc.gpsimd.tensor_tensor(Un[0:M], AB1[0:M], AB2[0:M], Alu.add)
                nc.vector.scalar_tensor_tensor(
                    Vn[0:M], pS[0:M], 2.0 / 400, pq[0:M], Alu.mult, Alu.mult
                )
                nc.vector.tensor_tensor(N1[0:M], Un[0:M], Vn[0:M], Alu.add)
                nc.vector.tensor_tensor(rtd1[0:M], RTb[0:M], d1[0:M], Alu.mult)
                nc.vector.tensor_tensor(PRn[0:M], rtd1[0:M], N1[0:M], Alu.subtract)
                nc.vector.tensor_tensor(MNn[0:M], RTb[0:M], N1[0:M], Alu.add)

                # denominators: fp32 Ln/Exp chain on ACT
                ul = den.tile([128, W], f32, tag="ul")
                vl = den.tile([128, W], f32, tag="vl")
                lnm1 = den.tile([128, W], f32, tag="lnm1")
                m1x = den.tile([128, W], f32, tag="m1x")
                Lp = den.tile([128, W], f32, tag="Lp")
                invp = den.tile([128, W], bf16, tag="invp")
                d15x = den.tile([128, W], f32, tag="d15x")
                Ll = den.tile([128, W], f32, tag="Ll")
                invl = den.tile([128, W], bf16, tag="invl")
                invm = den.tile([128, W], bf16, tag="invm")
                nc.scalar.activation(ul[0:M], d1[0:M], Act.Ln, bias=cbias("delta", M))
                nc.scalar.activation(vl[0:M], d1[0:M], Act.Ln, bias=cbias("one", M))
                nc.vector.scalar_tensor_tensor(
                    lnm1[0:M], vl[0:M], 0.5, ul[0:M], Alu.mult, Alu.add
                )
                nc.scalar.activation(m1x[0:M], lnm1[0:M], Act.Exp)
                nc.scalar.activation(Lp[0:M], m1x[0:M], Act.Ln, bias=cbias("eps", M))
                nc.scalar.activation(invp[0:M], Lp[0:M], Act.Exp, scale=-1.0)
                nc.scalar.activation(d15x[0:M], ul[0:M], Act.Exp, scale=1.5)
                nc.scalar.activation(Ll[0:M], d15x[0:M], Act.Ln, bias=cbias("eps", M))
                nc.scalar.activation(invl[0:M], Ll[0:M], Act.Exp, scale=-1.0)
                nc.scalar.activation(
                    invm[0:M], vl[0:M], Act.Exp, scale=-1.5, bias=cbias("lnhalf", M)
                )

                prof = outp.tile([128, W], bf16, tag="prof")
                plan = outp.tile([128, W], bf16, tag="plan")
                mean = outp.tile([128, W], bf16, tag="mean")
                nc.vector.tensor_tensor(prof[0:M], PRn[0:M], invp[0:M], Alu.mult)
                nc.vector.tensor_tensor(plan[0:M], N1[0:M], invl[0:M], Alu.mult)
                nc.vector.tensor_tensor(mean[0:M], MNn[0:M], invm[0:M], Alu.mult)
                return prof, plan, mean

            pair_idx = 0
            for s in range(B_PER_CORE):
                for blk in _BLOCKS:
                    M = blk[3]
                    outs_p = unit(pred_d, s, blk)
                    outs_t = unit(targ_d, s, blk)
                    for ci in range(3):
                        dp = outp.tile([128, W], bf16, tag="dp")
                        dead = outp.tile([128, W], bf16, tag="dp")
                        nc.gpsimd.tensor_tensor(
                            dp[0:M], outs_p[ci][0:M], outs_t[ci][0:M], Alu.subtract
                        )
                        col = pair_idx * 3 + ci
                        nc.scalar.activation(
                            dead[0:M],
                            dp[0:M],
                            Act.Abs,
                            accum_out=accbuf[0:M, col : col + 1],
                        )
                    pair_idx += 1

            nc.sync.dma_start(acc_d[:], accbuf[:])

    if split:
        _split_multi_waits(nc, mybir)
    return nc


def _split_multi_waits(nc, mybir):
    """This environment's walrus accepts only ONE semaphore wait per
    instruction (ISA Events has a single wait slot); Tile fuses several.
    Split every multi-wait instruction into standalone single-wait
    EventSemaphore ops on the same engine queue (raw-bass wait_ge style),
    keeping the last wait + all updates on the real instruction."""
    cnt = 0
    for f in nc.m.functions:
        for blk in f.blocks:
            new_list = []
            for ins in blk.instructions:
                si = ins.sync_info
                waits = list(si.on_wait) if si is not None else []
                if len(waits) > 1:
                    for w in waits[:-1]:
                        cnt += 1
                        nop = mybir.InstEventSemaphore(
                            name=f"wsplit-{cnt}", ins=[], outs=[]
                        )
                        nop.engine = ins.engine
                        nop.sync_info = mybir.SyncInfo(on_wait=[w], on_update=[])
                        new_list.append(nop)
                    si.on_wait = [waits[-1]]
                new_list.append(ins)
            blk.instructions = new_list


_NC_CACHE = {}


def _get_nc():
    if "nc" not in _NC_CACHE:
        _NC_CACHE["nc"] = _build_nc()
    return _NC_CACHE["nc"]


def kernel(pred, target):
    pred = np.ascontiguousarray(np.asarray(pred), dtype=np.float32)
    target = np.ascontiguousarray(np.asarray(target), dtype=np.float32)
    assert pred.shape == (B, 1, H, W) and target.shape == (B, 1, H, W)

    from concourse.bass_utils import run_bass_kernel_spmd

    nc = _get_nc()
    bands = _make_bands()
    consts = _make_consts()
    in_maps = [
        {
            "pred": pred[c * B_PER_CORE : (c + 1) * B_PER_CORE],
            "target": target[c * B_PER_CORE : (c + 1) * B_PER_CORE],
            "bands": bands,
            "consts": consts,
        }
        for c in range(N_CORES)
    ]
    res = run_bass_kernel_spmd(nc, in_maps, list(range(N_CORES)))
    _NC_CACHE["last_results"] = res

    tot = np.zeros(3, np.float64)
    for c in range(N_CORES):
        acc = np.asarray(res.results[c]["acc"], np.float64)
        for ci in range(3):
            tot[ci] += acc[:, ci : N_PAIR_UNITS * 3 : 3].sum()
    n = float(B * H * W)
    loss = (W_PROFILE * tot[0] + W_PLANFORM * tot[1] + W_MEAN * tot[2]) / n
    return np.float32(loss)
